# revision 52
# baseline (speedup 1.0000x reference)
"""ALiBi multi-head attention on 8 TRN2 NeuronCores.

Sharding: batch (2) x head-slots (4) = 8 cores. Core c handles batch b=c//4
and heads {4k + c%4 : k in 0..3} (one head per slope-group so the per-slot
ALiBi relevance windows are balanced and identical across cores - SPMD needs
one program). After attention, one 8-way AllToAll per i-group redistributes
head-shards -> sequence-shards: core c receives, from every core j, core j's
head-slice for i-64-chunk c of the i-group. Each core then runs the full
out-projection for its (batch0 + batch1) sequence slices, and the host
assembles the 8 outputs.

Math notes:
 - softmax_j(s_ij + c_i) == softmax_j(s_ij) for any per-row constant c_i, and
   our denominator comes from an appended ones-column in V, so the per-i
   "stabilizer" row only has to keep exp() in range - it does not need to be
   exact. We use bf16(-8*slope*i) so the matmul contributions are exact.
 - The ALiBi j-term and padding-mask term ride in the Q/K contraction as two
   extra rows, split hi/lo in bf16 so the sum is accurate to ~2^-18 rel.
 - scores psum = qk + 8*slope*(j - i) + mask; exp applies scale=1/8.
 - Blocks with i - j > 28/slope contribute < e^-28 relative and are skipped.
"""

import math
import os
import sys

import numpy as np

for _p in ("/opt/trn_rl_repo",):
    if _p not in sys.path:
        sys.path.append(_p)

B, S, DM = 2, 2048, 1024
H, D = 16, 64
N_CORES = 8
GROUPS = [[0, 1, 2, 3], [4, 5, 6, 7]]

IC = 256          # i-chunk width for attention (8 chunks)
N_ICS = S // IC
JB = 128          # j-block width (16 blocks)
N_JBS = S // JB
CHUNK_JBS = 4     # j-blocks per psum/exp chunk
MASK_NEG = -4000.0  # pre-scale (x8) additive mask -> exp(-500) == 0.0
WIN = 28.0        # ALiBi relevance radius in logits: drop i-j > WIN/slope

_NC = None        # cached compiled graph
LAST_RESULTS = None


def _slopes():
    start = 2.0 ** (-(2.0 ** -1.0))
    return np.array([start ** (h + 1) for h in range(H)], dtype=np.float64)


def _slot_windows():
    # jb-block window per slot, from the smallest slope in the slot (h=4s+3).
    slopes = _slopes()
    wins = []
    for s in range(4):
        r = WIN / slopes[4 * s + 3]
        wins.append(r)
    return wins


def _jb_range(s_win, ic):
    """Descending list of j-blocks for i-chunk ic given window radius."""
    jb_hi = (IC * (ic + 1) - 1) // JB          # = 2*ic + 1
    jb_lo = max(0, math.ceil((IC * ic - (JB - 1) - s_win) / JB))
    return list(range(jb_hi, jb_lo - 1, -1))


def _build():
    from concourse import bacc, bass, tile

    mybir = bass.mybir
    f32 = mybir.dt.float32
    bf16 = mybir.dt.bfloat16

    nc = bacc.Bacc("TRN2", target_bir_lowering=False, debug=False, num_devices=N_CORES)

    # ---- dram parameters (per-core shards supplied via in_maps) ----
    xT_d = nc.dram_tensor("xT", [DM, S], bf16, kind="ExternalInput").ap()
    w_d = nc.dram_tensor("wqkv", [DM, 768], bf16, kind="ExternalInput").ap()
    qaug_d = nc.dram_tensor("q_aug", [4, 3, S], bf16, kind="ExternalInput").ap()
    kaug_d = nc.dram_tensor("k_aug", [4, 3, S], bf16, kind="ExternalInput").ap()
    bqk_d = nc.dram_tensor("bias_qk", [64, 8], f32, kind="ExternalInput").ap()
    bv_d = nc.dram_tensor("bias_v", [128, 2], f32, kind="ExternalInput").ap()
    wout_d = nc.dram_tensor("wout", [DM, DM], bf16, kind="ExternalInput").ap()
    bout_d = nc.dram_tensor("bias_out", [128, 8], f32, kind="ExternalInput").ap()
    # out[n, b, ig, il]: global i = ig*512 + c*64 + il
    out_d = nc.dram_tensor("out", [DM, 2, 4, 64], f32, kind="ExternalOutput").ap()
    dbg = bool(os.environ.get("KERNEL_DEBUG"))
    if dbg:
        dbg_qk_d = nc.dram_tensor("dbg_qk", [8, 67, S], bf16, kind="ExternalOutput").ap()
        dbg_v_d = nc.dram_tensor("dbg_v", [128, N_JBS, 260], bf16, kind="ExternalOutput").ap()
        dbg_cat_d = nc.dram_tensor("dbg_cat", [2, 128, S], bf16, kind="ExternalOutput").ap()

    # ---- inline constants (shared across cores, baked into the NEFF) ----
    jl = np.arange(JB)[:, None]
    tri_f = np.where(jl > np.arange(JB)[None, :], np.float32(MASK_NEG), np.float32(0.0))
    all_f = np.full((JB, JB), np.float32(MASK_NEG), dtype=np.float32)
    zero_f = np.zeros((JB, JB), dtype=np.float32)
    m_tri_np = np.concatenate([all_f, tri_f, tri_f, zero_f], axis=1)  # [128, 512]
    tri01 = np.where(jl > np.arange(JB)[None, :], 0.0, 1.0)
    m01_np = np.concatenate(
        [np.zeros((JB, JB)), tri01, tri01, np.ones((JB, JB))], axis=1
    ).astype(np.float32)  # cast to bf16 on load
    import ml_dtypes

    m_tri_d = nc.inline_tensor(m_tri_np, name="m_tri").ap()
    m01_d = nc.inline_tensor(m01_np.astype(ml_dtypes.bfloat16), name="m01").ap()
    ident_d = nc.inline_tensor(np.eye(128, dtype=ml_dtypes.bfloat16), name="ident").ap()

    wins = _slot_windows()

    with tile.TileContext(nc) as tc:
        from contextlib import ExitStack

        with ExitStack() as stack:
            ep = stack.enter_context  # persistent pools

            # ------- persistent SBUF -------
            qk_pool = ep(tc.tile_pool(name="qk", bufs=1))
            qT = [qk_pool.tile([67, S], bf16, name=f"qT{s}") for s in range(4)]
            kT = [qk_pool.tile([67, S], bf16, name=f"kT{s}") for s in range(4)]
            vaug_pool = ep(tc.tile_pool(name="vaug", bufs=1))
            vaug = vaug_pool.tile([128, N_JBS, 260], bf16)
            cat_pool = ep(tc.tile_pool(name="cat", bufs=1))
            catT = [cat_pool.tile([128, S], bf16, name=f"catT{t}") for t in range(2)]
            # gathered full-d' tiles, per batch: catTf[b][:, m, :] is d'-chunk
            # m, free = 4 igroups x 64 local i-cols
            catTf = [cat_pool.tile([128, 8, 256], bf16, name=f"catTf{b}") for b in range(2)]
            const_pool = ep(tc.tile_pool(name="consts", bufs=1))
            m_tri = const_pool.tile([128, 512], f32, name="m_tri_sb")
            m01 = const_pool.tile([128, 512], bf16, name="m01_sb")
            ident = const_pool.tile([128, 128], bf16, name="ident_sb")
            bqk = const_pool.tile([64, 8], f32, name="bqk_sb")
            bv = const_pool.tile([128, 2], f32, name="bv_sb")
            bout = const_pool.tile([128, 8], f32, name="bout_sb")
            wout_pool = ep(tc.tile_pool(name="wout", bufs=1))
            wout = wout_pool.tile([128, 8, DM], bf16)

            # warm up the collective stream immediately: the first collective
            # op in a NEFF pays ~90us of entry-barrier/init - hide it here.
            dram_w = ep(tc.tile_pool(name="dramw", bufs=1, space="DRAM"))
            warm_a = dram_w.tile([8, 16], bf16)
            warm_b = dram_w.tile([8, 16], bf16)
            nc.gpsimd.collective_compute(
                "AllToAll",
                mybir.AluOpType.bypass,
                replica_groups=[list(range(8))],
                ins=[warm_a.opt()],
                outs=[warm_b.opt()],
            )

            # ------- phase 1: load x + qkv projection -------
            with ExitStack() as ph1:
                xt_pool = ph1.enter_context(tc.tile_pool(name="xt", bufs=1))
                xt = xt_pool.tile([128, 8, S], bf16)
                wq_pool = ph1.enter_context(tc.tile_pool(name="wq", bufs=1))
                wsb = wq_pool.tile([128, 8, 768], bf16)
                psp = ph1.enter_context(tc.tile_pool(name="psp", bufs=2, space="PSUM"))

                for k in range(8):
                    nc.sync.dma_start(out=xt[:, k, :], in_=xT_d[k * 128 : (k + 1) * 128, :])
                    nc.sync.dma_start(out=wsb[:, k, :], in_=w_d[k * 128 : (k + 1) * 128, :])

                # aug rows: qT rows 64-66 = [-8*slope*i, 1, 1] (stabilizer)
                #           kT rows 64-66 = [1, hi, lo] of 8*(slope*j + mask)
                for s in range(4):
                    nc.sync.dma_start(out=qT[s][64:67, :], in_=qaug_d[s, :, :])
                    nc.sync.dma_start(out=kT[s][64:67, :], in_=kaug_d[s, :, :])

                # bulk prefetch for later phases on the scalar engine's queue
                nc.sync.dma_start(out=bqk[:], in_=bqk_d[:])
                nc.scalar.dma_start(out=m_tri[:], in_=m_tri_d[:])
                nc.scalar.dma_start(out=m01[:], in_=m01_d[:])
                nc.scalar.dma_start(out=ident[:], in_=ident_d[:])
                nc.scalar.dma_start(out=bv[:], in_=bv_d[:])
                nc.scalar.dma_start(out=bout[:], in_=bout_d[:])
                for m in range(8):
                    nc.scalar.dma_start(
                        out=wout[:, m, :], in_=wout_d[m * 128 : (m + 1) * 128, :]
                    )

                # k then q waves (8 psum tiles each, k-outer inside a wave)
                # tile-major k-loops: chains complete staggered, so evacs
                # stream and the next wave's psum frees early (keeps PE dense
                # across wave boundaries -> HAM stays at full clock).
                for wave in range(2):  # 0 -> k, 1 -> q
                    ps = [psp.tile([128, 512], f32, name=f"pspt{i % 4}") for i in range(8)]
                    col0 = 256 if wave == 0 else 0  # k cols at 256:512, q at 0:256
                    for g in range(2):  # slot-pair column group
                        for it in range(4):
                            for k in range(8):
                                nc.tensor.matmul(
                                    ps[g * 4 + it][:],
                                    lhsT=wsb[:, k, col0 + g * 128 : col0 + (g + 1) * 128],
                                    rhs=xt[:, k, it * 512 : (it + 1) * 512],
                                    start=(k == 0),
                                    stop=(k == 7),
                                )
                    dst = kT if wave == 0 else qT
                    bcol0 = 4 if wave == 0 else 0
                    for g in range(2):
                        for it in range(4):
                            for h2 in range(2):
                                s = g * 2 + h2
                                if wave == 1:
                                    nc.scalar.activation(
                                        dst[s][0:64, it * 512 : (it + 1) * 512],
                                        ps[g * 4 + it][h2 * 64 : (h2 + 1) * 64, :],
                                        mybir.ActivationFunctionType.Identity,
                                        bias=bqk[:, bcol0 + s : bcol0 + s + 1],
                                    )
                                else:
                                    nc.vector.tensor_scalar_add(
                                        dst[s][0:64, it * 512 : (it + 1) * 512],
                                        ps[g * 4 + it][h2 * 64 : (h2 + 1) * 64, :],
                                        bqk[:, bcol0 + s : bcol0 + s + 1],
                                    )

                # v wave last: natural layout [j, d], 4 slots packed (256 cols)
                # NOTE: matmul start=True clears the whole PSUM *bank*, so in a
                # bank shared by two chains only the first-emitted chain may
                # set start=True; the second chain's first matmul overwrites
                # (its has_written bits were cleared by the first chain).
                ps = [psp.tile([128, 512], f32, name=f"pspt{i % 4}") for i in range(8)]
                for pair in range(8):
                    for k in range(8):
                        for half in range(2):
                            jb = pair * 2 + half
                            nc.tensor.matmul(
                                ps[pair][:, half * 256 : (half + 1) * 256],
                                lhsT=xt[:, k, jb * 128 : (jb + 1) * 128],
                                rhs=wsb[:, k, 512:768],
                                start=(k == 0 and half == 0),
                                stop=(k == 7),
                                skip_group_check=True,
                            )
                    for half in range(2):
                        jb = pair * 2 + half
                        for s in range(4):
                            nc.vector.tensor_copy(
                                vaug[:, jb, s * 65 : s * 65 + 64],
                                ps[pair][:, half * 256 + s * 64 : half * 256 + (s + 1) * 64],
                            )
                for s in range(4):
                    nc.vector.memset(vaug[:, :, s * 65 + 64 : s * 65 + 65], 1.0)

            # ------- phase 2: attention -------
            with ExitStack() as ph2:
                pssc = ph2.enter_context(tc.tile_pool(name="pssc", bufs=2, space="PSUM"))
                psa = ph2.enter_context(tc.tile_pool(name="psa", bufs=2, space="PSUM"))
                pst = ph2.enter_context(tc.tile_pool(name="pst", bufs=2, space="PSUM"))
                spt_pool = ph2.enter_context(tc.tile_pool(name="spt", bufs=3))
                anat_pool = ph2.enter_context(tc.tile_pool(name="anat", bufs=3))
                rec_pool = ph2.enter_context(tc.tile_pool(name="rec", bufs=4))
                osb_pool = ph2.enter_context(tc.tile_pool(name="osb", bufs=2))
                dram = ph2.enter_context(tc.tile_pool(name="dramp", bufs=1, space="DRAM"))

                # per-igroup A2A: shard j = [256 d', 64 i]
                a2a_in = [dram.tile([8, 256, 64], bf16, name=f"a2ain{i}") for i in range(4)]
                a2a_out = [dram.tile([8, 256, 64], bf16, name=f"a2aout{i}") for i in range(4)]

                def outproj(ig):
                    for n in range(8):
                        po = pst.tile([128, 128], f32, name="pt")
                        for m in range(8):
                            for bb in range(2):
                                nc.tensor.matmul(
                                    po[:, bb * 64 : (bb + 1) * 64],
                                    lhsT=wout[:, m, n * 128 : (n + 1) * 128],
                                    rhs=catTf[bb][:, m, ig * 64 : (ig + 1) * 64],
                                    start=(m == 0 and bb == 0),
                                    stop=(m == 7),
                                    skip_group_check=True,
                                )
                        ot = osb_pool.tile([128, 128], f32, name="ot")
                        nc.vector.tensor_scalar_add(ot[:], po[:], bout[:, n : n + 1])
                        nc.sync.dma_start(
                            out=out_d[n * 128 : (n + 1) * 128, :, ig, :],
                            in_=ot[:].rearrange("p (b r) -> p b r", b=2),
                        )

                # medium chunks first (dense enough to keep the PE clock
                # warm), big ones in the middle (overlap the A2A stream),
                # small ones last (their A2A + outproj tail is short).
                ic_order = [3, 2, 5, 4, 7, 6, 1, 0]
                done_ics = set()
                trigger_seq = []
                for ic in ic_order:
                    anat = [
                        anat_pool.tile([128, 128], bf16, name=f"anat{sub}{p}")
                        for sub in range(2)
                        for p in range(2)
                    ]  # index: sub*2 + pair
                    for s in range(4):
                        jbs = _jb_range(wins[s], ic)
                        pa = psa.tile([128, 130], f32, name="pa")
                        first = True
                        for c0 in range(0, len(jbs), CHUNK_JBS):
                            chunk = jbs[c0 : c0 + CHUNK_JBS]
                            w = len(chunk) * 256
                            psc = pssc.tile([128, 1024], f32, name="psc")
                            for pos, jb in enumerate(chunk):
                                # pos 0/2 clear bank 0/1 of the chunk tile
                                nc.tensor.matmul(
                                    psc[:, pos * 256 : (pos + 1) * 256],
                                    lhsT=kT[s][0:67, jb * 128 : (jb + 1) * 128],
                                    rhs=qT[s][0:67, ic * 256 : (ic + 1) * 256],
                                    start=(pos % 2 == 0),
                                    stop=True,
                                    skip_group_check=True,
                                )
                            if c0 == 0 and s == 0:
                                # slot 0 can overflow exp -> mask before exp
                                nc.vector.tensor_add(
                                    psc[:, 0:512], psc[:, 0:512], m_tri[:]
                                )
                            spt = spt_pool.tile([128, 1024], bf16, name="spt")
                            nc.scalar.activation(
                                spt[:, 0:w],
                                psc[:, 0:w],
                                mybir.ActivationFunctionType.Exp,
                                scale=0.125,
                            )
                            if c0 == 0 and s != 0:
                                nc.vector.tensor_mul(
                                    spt[:, 0:512], spt[:, 0:512], m01[:]
                                )
                            for pos, jb in enumerate(chunk):
                                for sub in range(2):
                                    nc.tensor.matmul(
                                        pa[:, sub * 65 : sub * 65 + 65],
                                        lhsT=spt[:, pos * 256 + sub * 128 : pos * 256 + (sub + 1) * 128],
                                        rhs=vaug[:, jb, s * 65 : (s + 1) * 65],
                                        start=(first and pos == 0 and sub == 0),
                                        stop=(c0 + CHUNK_JBS >= len(jbs) and pos == len(chunk) - 1),
                                        skip_group_check=True,
                                    )
                            first = False
                        # divide by the ones-column row-sums
                        r = rec_pool.tile([128, 2], f32, name="rcp")
                        nc.vector.reciprocal(
                            r[:], pa[:].rearrange("p (s c) -> p s c", s=2)[:, :, 64]
                        )
                        for sub in range(2):
                            nc.vector.tensor_scalar_mul(
                                anat[sub * 2 + s // 2][:, (s % 2) * 64 : (s % 2 + 1) * 64],
                                pa[:, sub * 65 : sub * 65 + 64],
                                r[:, sub : sub + 1],
                            )
                    # transpose [i, d'] -> [d', i] and land in catT with v-bias
                    for sub in range(2):
                        for p in range(2):
                            pt = pst.tile([128, 128], bf16, name="pt")
                            nc.tensor.matmul(
                                pt[:], lhsT=anat[sub * 2 + p][:], rhs=ident[:],
                                is_transpose=True,
                            )
                            nc.vector.tensor_scalar_add(
                                catT[p][:, ic * 256 + sub * 128 : ic * 256 + (sub + 1) * 128],
                                pt[:],
                                bv[:, p : p + 1],
                            )
                    done_ics.add(ic)
                    if (ic ^ 1) in done_ics:
                        # trigger this igroup's A2A; run the out-projection for
                        # the PREVIOUSLY-triggered igroup (its A2A completed
                        # during the attention work just done - no PE stall).
                        ig = ic // 2
                        for t in range(2):
                            nc.sync.dma_start(
                                out=a2a_in[ig][:, t * 128 : (t + 1) * 128, :].rearrange(
                                    "j p r -> p j r"
                                ),
                                in_=catT[t][:, ig * 512 : (ig + 1) * 512],
                            )
                        nc.gpsimd.collective_compute(
                            "AllToAll",
                            mybir.AluOpType.bypass,
                            replica_groups=[list(range(8))],
                            ins=[a2a_in[ig].opt()],
                            outs=[a2a_out[ig].opt()],
                        )
                        for bb in range(2):
                            nc.sync.dma_start(
                                out=catTf[bb][:, :, ig * 64 : (ig + 1) * 64],
                                in_=a2a_out[ig][4 * bb : 4 * bb + 4, :, :].rearrange(
                                    "j (h p) r -> p j h r", h=2
                                ),
                            )
                        if trigger_seq:
                            outproj(trigger_seq[-1])
                        trigger_seq.append(ig)

                outproj(trigger_seq[-1])

            if dbg:
                for s in range(4):
                    nc.sync.dma_start(out=dbg_qk_d[s, :, :], in_=qT[s][0:67, :])
                    nc.sync.dma_start(out=dbg_qk_d[4 + s, :, :], in_=kT[s][0:67, :])
                nc.sync.dma_start(out=dbg_v_d[:], in_=vaug[:])
                for t in range(2):
                    nc.sync.dma_start(out=dbg_cat_d[t, :, :], in_=catT[t][:])

    nc.compile()
    return nc


def _get_nc():
    global _NC
    if _NC is None:
        _NC = _build()
    return _NC


def kernel(**inputs):
    import ml_dtypes

    bf16 = ml_dtypes.bfloat16
    x = np.asarray(inputs["x"], dtype=np.float32)
    mask = np.asarray(inputs["attention_mask"])
    w_qkv = np.asarray(inputs["W_qkv"], dtype=np.float32)
    b_qkv = np.asarray(inputs["b_qkv"], dtype=np.float32)
    w_out = np.asarray(inputs["W_out"], dtype=np.float32)
    b_out = np.asarray(inputs["b_out"], dtype=np.float32)

    slopes = _slopes()
    j_idx = np.arange(S, dtype=np.float64)

    # wout rows permuted to the A2A arrival order: j-major, then slot k
    wout_perm = np.concatenate(
        [w_out[(4 * k + j) * 64 : (4 * k + j + 1) * 64, :] for j in range(4) for k in range(4)],
        axis=0,
    ).astype(bf16)
    bias_out = np.ascontiguousarray(b_out.reshape(8, 128).T).astype(np.float32)

    in_maps = []
    for c in range(N_CORES):
        b, g = divmod(c, 4)
        heads = [4 * k + g for k in range(4)]
        xT = np.ascontiguousarray(x[b].T).astype(bf16)
        cols = (
            [w_qkv[:, h * 64 : (h + 1) * 64] for h in heads]
            + [w_qkv[:, DM + h * 64 : DM + (h + 1) * 64] for h in heads]
            + [w_qkv[:, 2 * DM + h * 64 : 2 * DM + (h + 1) * 64] for h in heads]
        )
        wqkv = np.concatenate(cols, axis=1).astype(bf16)

        ones = np.ones(S, dtype=np.float64)
        q_aug = np.stack(
            [np.stack([-8.0 * slopes[h] * j_idx, ones, ones]) for h in heads]
        ).astype(bf16)
        mvec = -2000.0 * (1.0 - mask[b].astype(np.float64))
        k_rows = []
        for h in heads:
            v = 8.0 * (slopes[h] * j_idx + mvec)
            hi = v.astype(bf16)
            lo = (v - hi.astype(np.float64)).astype(bf16)
            k_rows.append(np.stack([ones.astype(bf16), hi, lo]))
        k_aug = np.stack(k_rows)

        bias_qk = np.stack(
            [b_qkv[h * 64 : (h + 1) * 64] for h in heads]
            + [b_qkv[DM + h * 64 : DM + (h + 1) * 64] for h in heads],
            axis=1,
        ).astype(np.float32)
        bias_v = np.stack(
            [
                np.concatenate(
                    [b_qkv[2 * DM + heads[2 * p] * 64 : 2 * DM + (heads[2 * p] + 1) * 64],
                     b_qkv[2 * DM + heads[2 * p + 1] * 64 : 2 * DM + (heads[2 * p + 1] + 1) * 64]]
                )
                for p in range(2)
            ],
            axis=1,
        ).astype(np.float32)

        in_maps.append(
            {
                "xT": xT,
                "wqkv": wqkv,
                "q_aug": np.ascontiguousarray(q_aug),
                "k_aug": np.ascontiguousarray(k_aug),
                "bias_qk": np.ascontiguousarray(bias_qk),
                "bias_v": np.ascontiguousarray(bias_v),
                "wout": wout_perm,
                "bias_out": bias_out,
            }
        )

    from concourse.bass_utils import run_bass_kernel_spmd

    nc = _get_nc()
    trace = bool(os.environ.get("KERNEL_TRACE"))
    tmpdir = os.environ.get("KERNEL_TMPDIR") or None
    res = run_bass_kernel_spmd(
        nc, in_maps, core_ids=list(range(N_CORES)), trace=trace, tmpdir=tmpdir
    )
    global LAST_RESULTS
    LAST_RESULTS = res

    out = np.empty((B, S, DM), dtype=np.float32)
    for c in range(N_CORES):
        arr = res.results[c]["out"]  # [1024 n, 2 b, 4 ig, 64 il]
        for bb in range(B):
            for ig in range(4):
                out[bb, ig * 512 + c * 64 : ig * 512 + (c + 1) * 64, :] = arr[
                    :, bb, ig, :
                ].T
    return out
